# revision 2
# baseline (speedup 1.0000x reference)
"""GemmaAttention (GQA, B=2 S=2048 HID=2048, 16 q-heads / 4 kv-heads, d=256)
on 8 Trainium2 NeuronCores.

Sharding: core = (batch b, head-group g) with b = core//4, g = core%4.
Each core computes q-heads [4g, 4g+4) and kv-head g (the reference's
repeat_kv quirk maps q-head h to kv-head h//4), producing a partial
o_proj output [S, HID] from its 1024 o_proj input features.  The host
sums the 4 partials per batch.  No collectives.

All GEMMs run on the PE in fp8-e4m3 DoubleRow (double-pumped) mode with
hi/lo split-precision operands: x ~= (x_hi + x_lo)/s with both parts
e4m3 at a shared power-of-2 scale s.  Each K=256 slab is then three
DoubleRow matmuls (hi*hi + hi*lo + lo*hi; the lo*lo term is ~eps^2 and
dropped), i.e. 0.75x the bf16 cycle cost at slightly BETTER than bf16
accuracy (measured 2.5e-3 vs 5.1e-3 end-to-end).  Weight/hs splits are
prepared host-side; q/k (rope output), v, probs (exp output) and o are
split on-chip with one extra copy+sub per tile.

Scales: hs 32, W* 1024, rope tables pre-scaled 2^-11 so q/k land at 16,
v rescaled to 32 at PSUM evac, probs at 1, o at 32 (falls out of the
1/rowsum multiply), final out copy descales by 2^-15.  All on-chip fp8
writes have >=2x headroom to e4m3's +-240 (probs max ~205 of 240 on the
fixed harness input).

On-chip layout is "transposed" throughout: hsT [HID, S], qT/kT [d, S],
v natural [S, d], scores computed transposed [ks, qs].  Softmax skips
max-subtraction (exp input is O(5), cannot overflow); 1/(sqrt(d)*256)
is folded into exp's scale immediate; the causal triangle mask is
accumulated into the scores PSUM via a bf16 identity matmul that also
opens the accumulation group on diagonal tiles, whose columns are
sliced to their live range.  PV pairs adjacent key-tiles into single
DoubleRow matmuls; the masked gap columns of the odd plane of a
diagonal pair are memset to zero.
"""

import sys

sys.path.insert(0, "/opt/trn_rl_repo")

import math

import numpy as np
import ml_dtypes

import concourse.bacc as bacc
import concourse.bass as bass
import concourse.bass_isa as bass_isa
import concourse.tile as tile
from concourse import mybir
from concourse.bass_utils import run_bass_kernel_spmd

B, S, HID = 2, 2048, 2048
N_HEADS, N_KV, HEAD_DIM = 16, 4, 256
HD2 = HEAD_DIM // 2  # 128
ROPE_BASE = 10000.0
P = 128
QB = 512  # qs block width (moving free dim)
NSB = S // QB  # 4 s-blocks
NHT = HID // P  # 16 hidden chunks
NKS = S // P  # 16 key tiles
HPC = N_HEADS // 4  # 4 q heads per core
FQ = HPC * HEAD_DIM  # 1024 q features per core
NFQ = FQ // P  # 8 qT partition tiles
SCALE = 1.0 / math.sqrt(HEAD_DIM)

F32 = mybir.dt.float32
BF16 = mybir.dt.bfloat16
F8 = mybir.dt.float8e4
NP_BF16 = ml_dtypes.bfloat16
NP_F8 = ml_dtypes.float8_e4m3
F8MAX = 240.0
DR = mybir.MatmulPerfMode.DoubleRow

# power-of-2 quantization scales
S_HS = 32.0
S_W = 1024.0
S_QK = 16.0  # rope tables pre-scaled by S_QK / (S_HS * S_W) = 2^-11
S_V = 32.0
S_O = 32.0
EXP_SCALE = SCALE / (S_QK * S_QK)  # 2^-12
OUT_SCALE = 1.0 / (S_O * S_W)  # 2^-15
MASK_HUGE = -1.0e15


def _build(mask_mode: str):
    """mask_mode: 'causal' | 'none' | 'full'. Returns compiled Bacc."""
    nc = bacc.Bacc("TRN2", target_bir_lowering=False, debug=False, num_devices=8)

    hsT_hi = nc.dram_tensor("hsT_hi", [HID, S], F8, kind="ExternalInput").ap()
    hsT_lo = nc.dram_tensor("hsT_lo", [HID, S], F8, kind="ExternalInput").ap()
    wq_hi = nc.dram_tensor("wq_hi", [HID, FQ], F8, kind="ExternalInput").ap()
    wq_lo = nc.dram_tensor("wq_lo", [HID, FQ], F8, kind="ExternalInput").ap()
    wk_hi = nc.dram_tensor("wk_hi", [HID, HEAD_DIM], F8, kind="ExternalInput").ap()
    wk_lo = nc.dram_tensor("wk_lo", [HID, HEAD_DIM], F8, kind="ExternalInput").ap()
    wv_hi = nc.dram_tensor("wv_hi", [HID, HEAD_DIM], F8, kind="ExternalInput").ap()
    wv_lo = nc.dram_tensor("wv_lo", [HID, HEAD_DIM], F8, kind="ExternalInput").ap()
    wo_hi = nc.dram_tensor("wo_hi", [FQ, HID], F8, kind="ExternalInput").ap()
    wo_lo = nc.dram_tensor("wo_lo", [FQ, HID], F8, kind="ExternalInput").ap()
    cosT = nc.dram_tensor("cosT", [HD2, S], F32, kind="ExternalInput").ap()
    sinT = nc.dram_tensor("sinT", [HD2, S], F32, kind="ExternalInput").ap()
    if mask_mode == "causal":
        ident = nc.dram_tensor("ident", [P, P], BF16, kind="ExternalInput").ap()
        mtri = nc.dram_tensor("mtri", [P, P], BF16, kind="ExternalInput").ap()
    elif mask_mode == "full":
        ident = nc.dram_tensor("ident", [P, P], BF16, kind="ExternalInput").ap()
        maskT = nc.dram_tensor("maskT", [S, S], BF16, kind="ExternalInput").ap()
    out = nc.dram_tensor("out", [S, HID], F32, kind="ExternalOutput").ap()

    with tile.TileContext(nc) as tc:
        with (
            tc.tile_pool(name="resid", bufs=1) as resid,
            tc.tile_pool(name="hst", bufs=1) as hst_pool,
            tc.tile_pool(name="oT", bufs=2) as oT_pool,
            tc.tile_pool(name="probs", bufs=3) as probs_pool,
            tc.tile_pool(name="et", bufs=3) as et_pool,
            tc.tile_pool(name="tmp", bufs=1) as tmp_pool,
            tc.tile_pool(name="rb", bufs=1) as rb_pool,
            tc.tile_pool(name="mchunk", bufs=4) as mchunk_pool,
            tc.tile_pool(name="outsb", bufs=2) as outsb_pool,
            tc.tile_pool(name="wo", bufs=2) as wo_pool,
            tc.tile_pool(name="ot", bufs=2) as ot_pool,
            tc.tile_pool(name="mm_ps", bufs=3, space="PSUM") as mm_ps,
            tc.tile_pool(name="sc_ps", bufs=3, space="PSUM") as sc_ps,
            tc.tile_pool(name="o_ps", bufs=2, space="PSUM") as o_ps,
            tc.tile_pool(name="sacc", bufs=2) as sacc_pool,
        ):
            # ---- persistent tiles (hi/lo fp8 pairs; dim1 = k-subtile) ----
            qT_hi = [resid.tile([P, 2, S], F8, tag=f"qTh{i}", name=f"qTh{i}") for i in range(HPC)]
            qT_lo = [resid.tile([P, 2, S], F8, tag=f"qTl{i}", name=f"qTl{i}") for i in range(HPC)]
            kT_hi = resid.tile([P, 2, S], F8, tag="kTh", name="kTh")
            kT_lo = resid.tile([P, 2, S], F8, tag="kTl", name="kTl")
            vt_hi = resid.tile([P, NKS, HEAD_DIM], F8, tag="vh", name="vh")
            vt_lo = resid.tile([P, NKS, HEAD_DIM], F8, tag="vl", name="vl")
            cos_t = resid.tile([HD2, S], F32, tag="cos", name="cos_t")
            sin_t = resid.tile([HD2, S], F32, tag="sin", name="sin_t")
            wkh = resid.tile([P, NHT, HEAD_DIM], F8, tag="wkh", name="wkh")
            wkl = resid.tile([P, NHT, HEAD_DIM], F8, tag="wkl", name="wkl")
            wvh = resid.tile([P, NHT, HEAD_DIM], F8, tag="wvh", name="wvh")
            wvl = resid.tile([P, NHT, HEAD_DIM], F8, tag="wvl", name="wvl")
            wqh = resid.tile([P, NHT, FQ], F8, tag="wqh", name="wqh")
            wql = resid.tile([P, NHT, FQ], F8, tag="wql", name="wql")
            if mask_mode in ("causal", "full"):
                id_t = resid.tile([P, P], BF16, tag="ident", name="id_t")
                nc.sync.dma_start(out=id_t, in_=ident)
            if mask_mode == "causal":
                mt_t = resid.tile([P, P], BF16, tag="mtri", name="mt_t")
                nc.sync.dma_start(out=mt_t, in_=mtri)

            def dr3(ps, a_hi, a_lo, b_hi, b_lo, start, stop):
                """ps += a.T @ b over one K=256 slab (2 k-subtiles) via three
                fp8 DoubleRow matmuls, dropping the lo*lo term."""
                nc.tensor.matmul(ps, lhsT=a_hi, rhs=b_hi, start=start, stop=False, perf_mode=DR)
                nc.tensor.matmul(ps, lhsT=a_hi, rhs=b_lo, start=False, stop=False, perf_mode=DR)
                nc.tensor.matmul(ps, lhsT=a_lo, rhs=b_hi, start=False, stop=stop, perf_mode=DR)

            def rope_split(ps0, ps1, hiT, loT, sb):
                """RoPE + hi/lo fp8 split.  ps0/ps1: PSUM [P,QB] d-halves at
                scale S_HS*S_W; cos/sin pre-scaled so outputs land at S_QK.
                hiT/loT: [P, 2, S] fp8, planes = d-halves.  Reads of ps0
                first so its PSUM slot frees early."""
                cs = cos_t[:, sb * QB : (sb + 1) * QB]
                sn = sin_t[:, sb * QB : (sb + 1) * QB]
                t0 = tmp_pool.tile([P, QB], F32, tag="t0", name="t0")
                t1 = tmp_pool.tile([P, QB], F32, tag="t1", name="t1")
                t2 = tmp_pool.tile([P, QB], F32, tag="t2", name="t2")
                t3 = tmp_pool.tile([P, QB], F32, tag="t3", name="t3")
                d0 = tmp_pool.tile([P, QB], F32, tag="d0", name="d0")
                d1 = tmp_pool.tile([P, QB], F32, tag="d1", name="d1")
                nc.vector.tensor_mul(t0, ps0, cs)
                nc.vector.tensor_mul(t3, ps0, sn)
                nc.vector.tensor_mul(t1, ps1, sn)
                nc.vector.tensor_mul(t2, ps1, cs)
                sl = slice(sb * QB, (sb + 1) * QB)
                nc.vector.tensor_sub(d0, t0, t1)
                nc.vector.tensor_copy(out=hiT[:, 0, sl], in_=d0)
                nc.vector.tensor_sub(loT[:, 0, sl], d0, hiT[:, 0, sl])
                nc.vector.tensor_add(d1, t2, t3)
                nc.vector.tensor_copy(out=hiT[:, 1, sl], in_=d1)
                nc.vector.tensor_sub(loT[:, 1, sl], d1, hiT[:, 1, sl])

            # ================= phase 1: projections + rope =================
            for sb in range(NSB):
                ssl = slice(sb * QB, (sb + 1) * QB)
                hs_hi = hst_pool.tile([P, NHT, QB], F8, tag="hsh", name="hs_hi")
                hs_lo = hst_pool.tile([P, NHT, QB], F8, tag="hsl", name="hs_lo")
                for hq in range(4):
                    hsl4 = slice(4 * hq, 4 * hq + 4)
                    if sb == 0:
                        nc.sync.dma_start(
                            out=wkh[:, hsl4, :],
                            in_=wk_hi.rearrange("(t p) f -> p t f", p=P)[:, hsl4, :],
                        )
                        nc.sync.dma_start(
                            out=wkl[:, hsl4, :],
                            in_=wk_lo.rearrange("(t p) f -> p t f", p=P)[:, hsl4, :],
                        )
                    nc.sync.dma_start(
                        out=hs_hi[:, hsl4, :],
                        in_=hsT_hi.rearrange("(t p) s -> p t s", p=P)[:, hsl4, ssl],
                    )
                    nc.sync.dma_start(
                        out=hs_lo[:, hsl4, :],
                        in_=hsT_lo.rearrange("(t p) s -> p t s", p=P)[:, hsl4, ssl],
                    )
                # rope tables arrive as per-sb slices, keeping the startup
                # DMA window down to what the first kT/v matmuls need
                nc.sync.dma_start(out=cos_t[:, ssl], in_=cosT[:, ssl])
                nc.sync.dma_start(out=sin_t[:, ssl], in_=sinT[:, ssl])
                if sb == 0:
                    for hq in range(4):
                        hsl4 = slice(4 * hq, 4 * hq + 4)
                        nc.sync.dma_start(
                            out=wvh[:, hsl4, :],
                            in_=wv_hi.rearrange("(t p) f -> p t f", p=P)[:, hsl4, :],
                        )
                        nc.sync.dma_start(
                            out=wvl[:, hsl4, :],
                            in_=wv_lo.rearrange("(t p) f -> p t f", p=P)[:, hsl4, :],
                        )
                    # deferred bulk load: behind the sb0 essentials
                    for hq in range(4):
                        hsl4 = slice(4 * hq, 4 * hq + 4)
                        nc.sync.dma_start(
                            out=wqh[:, hsl4, :],
                            in_=wq_hi.rearrange("(t p) f -> p t f", p=P)[:, hsl4, :],
                        )
                        nc.sync.dma_start(
                            out=wql[:, hsl4, :],
                            in_=wq_lo.rearrange("(t p) f -> p t f", p=P)[:, hsl4, :],
                        )
                # k^T (one kv head: 2 d-halves), with rope
                ps_k = []
                for fd in range(2):
                    ps = mm_ps.tile([P, QB], F32, tag="mm", name="ps_mm")
                    fsl = slice(fd * P, (fd + 1) * P)
                    for tp in range(NHT // 2):
                        t2s = slice(2 * tp, 2 * tp + 2)
                        dr3(
                            ps,
                            wkh[:, t2s, fsl], wkl[:, t2s, fsl],
                            hs_hi[:, t2s, :], hs_lo[:, t2s, :],
                            start=(tp == 0), stop=(tp == NHT // 2 - 1),
                        )
                    ps_k.append(ps)
                rope_split(ps_k[0], ps_k[1], kT_hi, kT_lo, sb)
                # v (natural layout [s, d]), rescaled S_HS*S_W -> S_V at evac
                for s_sub in range(4):
                    pssl = slice(s_sub * P, (s_sub + 1) * P)
                    ps = mm_ps.tile([P, QB], F32, tag="mm", name="ps_mm")
                    for tp in range(NHT // 2):
                        t2s = slice(2 * tp, 2 * tp + 2)
                        dr3(
                            ps[:, :HEAD_DIM],
                            hs_hi[:, t2s, pssl], hs_lo[:, t2s, pssl],
                            wvh[:, t2s, :], wvl[:, t2s, :],
                            start=(tp == 0), stop=(tp == NHT // 2 - 1),
                        )
                    vi = sb * 4 + s_sub
                    vtmp = tmp_pool.tile([P, HEAD_DIM], F32, tag="vtmp", name="vtmp")
                    nc.vector.tensor_scalar_mul(vtmp, ps[:, :HEAD_DIM], S_V / (S_HS * S_W))
                    nc.vector.tensor_copy(out=vt_hi[:, vi, :], in_=vtmp)
                    nc.vector.tensor_sub(vt_lo[:, vi, :], vtmp, vt_hi[:, vi, :])
                # q^T (4 heads x 2 d-halves), with rope
                for h in range(HPC):
                    ps_q = []
                    for fd in range(2):
                        ft = 2 * h + fd
                        fsl = slice(ft * P, (ft + 1) * P)
                        ps = mm_ps.tile([P, QB], F32, tag="mm", name="ps_mm")
                        for tp in range(NHT // 2):
                            t2s = slice(2 * tp, 2 * tp + 2)
                            dr3(
                                ps,
                                wqh[:, t2s, fsl], wql[:, t2s, fsl],
                                hs_hi[:, t2s, :], hs_lo[:, t2s, :],
                                start=(tp == 0), stop=(tp == NHT // 2 - 1),
                            )
                        ps_q.append(ps)
                    rope_split(ps_q[0], ps_q[1], qT_hi[h], qT_lo[h], sb)

            # ============ phase 2+3: attention + o_proj, per qs-block ============
            for qb in range(NSB):
                qsl = slice(qb * QB, (qb + 1) * QB)
                nks = 4 * qb + 4 if mask_mode == "causal" else NKS
                oTh = oT_pool.tile([P, NFQ, QB], F8, tag="oTh", name="oTh")
                oTl = oT_pool.tile([P, NFQ, QB], F8, tag="oTl", name="oTl")
                for h in range(HPC):
                    ps_o0 = o_ps.tile([P, QB], F32, tag="o", name="ps_o")
                    ps_o1 = o_ps.tile([P, QB], F32, tag="o", name="ps_o")
                    acc = sacc_pool.tile([P, QB], F32, tag="acc", name="acc")
                    for pr in range(nks // 2):
                        ph = probs_pool.tile([P, 2, QB], F8, tag="ph", name="ph")
                        pl = probs_pool.tile([P, 2, QB], F8, tag="pl", name="pl")
                        c0s = []
                        for j in range(2):
                            ks = 2 * pr + j
                            ksl = slice(ks * P, (ks + 1) * P)
                            m = ks - 4 * qb if mask_mode == "causal" else -1
                            # columns [0, 128m) of a diagonal tile are fully
                            # masked -> compute only the live range [c0, QB)
                            c0 = 128 * m if m > 0 else 0
                            w = QB - c0
                            c0s.append(c0)
                            qslw = slice(qb * QB + c0, (qb + 1) * QB)
                            ps_s = sc_ps.tile([P, QB], F32, tag="sc", name="ps_s")
                            if m >= 0:
                                # triangle mask opens the accumulation group
                                # (start clears the bank; scores then overwrite
                                # the never-written columns, accumulate on the
                                # triangle ones)
                                nc.tensor.matmul(
                                    ps_s[:, c0 : c0 + P],
                                    lhsT=id_t,
                                    rhs=mt_t,
                                    start=True,
                                    stop=False,
                                )
                            dr3(
                                ps_s[:, c0:],
                                kT_hi[:, :, ksl], kT_lo[:, :, ksl],
                                qT_hi[h][:, :, qslw], qT_lo[h][:, :, qslw],
                                start=(m < 0), stop=(mask_mode != "full"),
                            )
                            if mask_mode == "full":
                                mc = mchunk_pool.tile([P, QB], BF16, tag="mc", name="mc")
                                nc.sync.dma_start(out=mc, in_=maskT[ksl, qsl])
                                nc.tensor.matmul(
                                    ps_s, lhsT=id_t, rhs=mc, start=False, stop=True
                                )
                            # exp in f32 (rowsum reads it), then fp8 hi/lo
                            et = acc if ks == 0 else et_pool.tile(
                                [P, QB], F32, tag="et", name="et"
                            )
                            nc.scalar.activation(
                                et[:, :w],
                                ps_s[:, c0:],
                                mybir.ActivationFunctionType.Exp,
                                scale=EXP_SCALE,
                            )
                            nc.vector.tensor_copy(out=ph[:, j, c0:], in_=et[:, :w])
                            nc.vector.tensor_sub(pl[:, j, c0:], et[:, :w], ph[:, j, c0:])
                            if ks > 0:
                                nc.vector.tensor_add(
                                    acc[:, c0:], acc[:, c0:], et[:, :w]
                                )
                        if c0s[1] > c0s[0]:
                            # odd plane of a diagonal pair: zero the masked gap
                            nc.any.memset(ph[:, 1, c0s[0] : c0s[1]], 0.0)
                            nc.any.memset(pl[:, 1, c0s[0] : c0s[1]], 0.0)
                        c0p = c0s[0]
                        k2s = slice(2 * pr, 2 * pr + 2)
                        for dh, ps_o in ((0, ps_o0), (1, ps_o1)):
                            dsl = slice(dh * HD2, (dh + 1) * HD2)
                            nc.tensor.matmul(
                                ps_o[:, c0p:],
                                lhsT=vt_hi[:, k2s, dsl],
                                rhs=ph[:, :, c0p:],
                                start=(pr == 0), stop=False, perf_mode=DR,
                            )
                            nc.tensor.matmul(
                                ps_o[:, c0p:],
                                lhsT=vt_hi[:, k2s, dsl],
                                rhs=pl[:, :, c0p:],
                                start=False, stop=False, perf_mode=DR,
                            )
                            nc.tensor.matmul(
                                ps_o[:, c0p:],
                                lhsT=vt_lo[:, k2s, dsl],
                                rhs=ph[:, :, c0p:],
                                start=False, stop=(pr == nks // 2 - 1), perf_mode=DR,
                            )
                    # 1/colsum: partition all-reduce (broadcasts too), recip
                    zb = rb_pool.tile([P, QB], F32, tag="zb", name="zb")
                    nc.gpsimd.partition_all_reduce(
                        zb, acc, channels=P, reduce_op=bass_isa.ReduceOp.add
                    )
                    rb = rb_pool.tile([P, QB], F32, tag="rb", name="rb")
                    nc.vector.reciprocal_approx_fast(rb, zb)
                    # evacuate o^T (frees PSUM), normalize, fp8 hi/lo split
                    for dh, ps_o in ((0, ps_o0), (1, ps_o1)):
                        f = 2 * h + dh
                        t = ot_pool.tile([P, QB], F32, tag="ot", name="ot")
                        nc.vector.tensor_copy(out=t, in_=ps_o)
                        nc.vector.tensor_mul(t, t, rb)
                        nc.vector.tensor_copy(out=oTh[:, f, :], in_=t)
                        nc.vector.tensor_sub(oTl[:, f, :], t, oTh[:, f, :])
                # ---- o_proj for this qs-block ----
                for hc in range(NSB):
                    hsl = slice(hc * QB, (hc + 1) * QB)
                    woh = wo_pool.tile([P, NFQ, QB], F8, tag="woh", name="woh")
                    wol = wo_pool.tile([P, NFQ, QB], F8, tag="wol", name="wol")
                    nc.sync.dma_start(
                        out=woh,
                        in_=wo_hi.rearrange("(t p) h -> p t h", p=P)[:, :, hsl],
                    )
                    nc.sync.dma_start(
                        out=wol,
                        in_=wo_lo.rearrange("(t p) h -> p t h", p=P)[:, :, hsl],
                    )
                    for s_sub in range(4):
                        pssl = slice(s_sub * P, (s_sub + 1) * P)
                        ps = mm_ps.tile([P, QB], F32, tag="mm", name="ps_mm")
                        for fp_ in range(NFQ // 2):
                            f2s = slice(2 * fp_, 2 * fp_ + 2)
                            dr3(
                                ps,
                                oTh[:, f2s, pssl], oTl[:, f2s, pssl],
                                woh[:, f2s, :], wol[:, f2s, :],
                                start=(fp_ == 0), stop=(fp_ == NFQ // 2 - 1),
                            )
                        ot = outsb_pool.tile([P, QB], F32, tag="ots", name="ots")
                        nc.scalar.activation(
                            ot, ps, mybir.ActivationFunctionType.Copy,
                            scale=OUT_SCALE,
                        )
                        nc.sync.dma_start(
                            out=out[qb * QB + s_sub * P : qb * QB + (s_sub + 1) * P, hsl],
                            in_=ot,
                        )
    nc.compile()
    return nc


_BUILD_CACHE: dict = {}


def _get_kernel(mask_mode: str):
    if mask_mode not in _BUILD_CACHE:
        _BUILD_CACHE[mask_mode] = _build(mask_mode)
    return _BUILD_CACHE[mask_mode]


def _rope_tables(position_ids_b: np.ndarray):
    """cos/sin half-tables, transposed [HD2, S] f32, pre-scaled by
    S_QK/(S_HS*S_W) so rope output lands at scale S_QK."""
    inv_freq = (
        1.0 / (ROPE_BASE ** (np.arange(0, HEAD_DIM, 2, dtype=np.float32) / HEAD_DIM))
    ).astype(np.float32)
    freqs = position_ids_b.astype(np.float32)[:, None] * inv_freq[None, :]  # [S, HD2]
    rs = np.float32(S_QK / (S_HS * S_W))
    return (
        np.ascontiguousarray((np.cos(freqs) * rs).astype(np.float32).T),
        np.ascontiguousarray((np.sin(freqs) * rs).astype(np.float32).T),
    )


def _split8(x: np.ndarray, scale: float):
    """x -> (hi, lo) e4m3 pair with x ~= (hi+lo)/scale."""
    xs = np.clip(x.astype(np.float32) * np.float32(scale), -F8MAX, F8MAX)
    hi = xs.astype(NP_F8)
    lo = np.clip(xs - hi.astype(np.float32), -F8MAX, F8MAX).astype(NP_F8)
    return np.ascontiguousarray(hi), np.ascontiguousarray(lo)


def kernel(hidden_states, attention_mask, position_ids, Wq, Wk, Wv, Wo):
    hidden_states = np.asarray(hidden_states, dtype=np.float32)
    attention_mask = np.asarray(attention_mask, dtype=np.float32)
    position_ids = np.asarray(position_ids)
    Wq = np.asarray(Wq, dtype=np.float32)
    Wk = np.asarray(Wk, dtype=np.float32)
    Wv = np.asarray(Wv, dtype=np.float32)
    Wo = np.asarray(Wo, dtype=np.float32)

    # mask classification
    tri = np.tril(np.ones((S, S), dtype=bool))
    canonical = np.where(tri, np.float32(0.0), np.float32(-1e9))
    is_causal = all(
        np.array_equal(attention_mask[b, 0], canonical) for b in range(B)
    )
    if is_causal:
        mask_mode = "causal"
    elif not attention_mask.any():
        mask_mode = "none"
    else:
        mask_mode = "full"

    nc = _get_kernel(mask_mode)

    ident = np.eye(P, dtype=np.float32).astype(NP_BF16)
    if mask_mode == "causal":
        ii = np.arange(P)[:, None]
        jj = np.arange(P)[None, :]
        mtri = np.where(jj >= ii, np.float32(0.0), np.float32(MASK_HUGE)).astype(NP_BF16)

    in_maps = []
    for core in range(8):
        b, g = core // 4, core % 4
        hsT_hi, hsT_lo = _split8(hidden_states[b].T, S_HS)
        wq_hi, wq_lo = _split8(Wq[:, g * FQ : (g + 1) * FQ], S_W)
        wk_hi, wk_lo = _split8(Wk[:, g * HEAD_DIM : (g + 1) * HEAD_DIM], S_W)
        wv_hi, wv_lo = _split8(Wv[:, g * HEAD_DIM : (g + 1) * HEAD_DIM], S_W)
        wo_hi, wo_lo = _split8(Wo[g * FQ : (g + 1) * FQ, :], S_W)
        m = {
            "hsT_hi": hsT_hi, "hsT_lo": hsT_lo,
            "wq_hi": wq_hi, "wq_lo": wq_lo,
            "wk_hi": wk_hi, "wk_lo": wk_lo,
            "wv_hi": wv_hi, "wv_lo": wv_lo,
            "wo_hi": wo_hi, "wo_lo": wo_lo,
        }
        cosT, sinT = _rope_tables(position_ids[b])
        m["cosT"], m["sinT"] = cosT, sinT
        if mask_mode == "causal":
            m["ident"] = ident
            m["mtri"] = mtri
        elif mask_mode == "full":
            m["ident"] = ident
            m["maskT"] = np.ascontiguousarray(
                (attention_mask[b, 0].T * np.float32(S_QK * S_QK))
            ).astype(NP_BF16)
        in_maps.append(m)

    global _LAST_IN_MAPS
    _LAST_IN_MAPS = in_maps
    res = run_bass_kernel_spmd(nc, in_maps, list(range(8)))
    outs = [res.results[c]["out"].astype(np.float32) for c in range(8)]
    full = np.empty((B, S, HID), dtype=np.float32)
    for b in range(B):
        full[b] = outs[4 * b] + outs[4 * b + 1] + outs[4 * b + 2] + outs[4 * b + 3]
    return full


# revision 13
# speedup vs baseline: 1.3374x; 1.3374x over previous
"""GemmaAttention (GQA, B=2 S=2048 HID=2048, 16 q-heads / 4 kv-heads, d=256)
on 8 Trainium2 NeuronCores.

Sharding: core = (batch b, head-group g) with b = core//4, g = core%4.
Each core computes q-heads [4g, 4g+4) and kv-head g (the reference's
repeat_kv quirk maps q-head h to kv-head h//4), producing a partial
o_proj output [S, HID] from its 1024 o_proj input features.  The host
sums the 4 partials per batch.  No collectives.

The big GEMMs (q/k/v projections, scores, o_proj) run on the PE in
fp8-e4m3 DoubleRow (double-pumped) mode with hi/lo split-precision
operands: x ~= (x_hi + x_lo)/s, both parts e4m3 at a shared power-of-2
scale.  Each K=256 slab is three DoubleRow matmuls (hi*hi + hi*lo +
lo*hi; lo*lo is ~eps^2 and dropped) = 0.75x the bf16 cycle cost at
better-than-bf16 accuracy.  hs and all weights are split host-side for
free; q/k are split on-chip after RoPE.  PV stays bf16: splitting probs
on-chip costs far more vector/scalar time than the PE time it saves.

Engine balance: exp and the PSUM->SBUF evacuation copies run on ACT;
RoPE multiplies and hi/lo splits on DVE (bf16 operands in SBUF for the
DVE fast path); rowsum partition-reduce on GPSIMD.

Scales: hs 32, W* 1024, rope tables pre-scaled 2^-11 so q/k land at 16,
v rescaled to 32 at PSUM evac, probs at 1, o at 32 (falls out of the
1/rowsum multiply), final out copy descales by 2^-15.  All on-chip fp8
writes have >=2x headroom to e4m3's +-240.

On-chip layout is "transposed" throughout: hsT [HID, S], qT/kT [d, S],
v natural [S, d], scores computed transposed [ks, qs].  Softmax skips
max-subtraction (exp input is O(5), cannot overflow); 1/(sqrt(d)*256)
is folded into exp's scale immediate; the causal triangle mask is
accumulated into the scores PSUM via a bf16 identity matmul that also
opens the accumulation group on diagonal tiles, whose columns are
sliced to their live range [128m, 512).
"""

import sys

sys.path.insert(0, "/opt/trn_rl_repo")

import math

import numpy as np
import ml_dtypes

import concourse.bacc as bacc
import concourse.bass as bass
import concourse.bass_isa as bass_isa
import concourse.tile as tile
from concourse import mybir
from concourse.bass_utils import run_bass_kernel_spmd

B, S, HID = 2, 2048, 2048
N_HEADS, N_KV, HEAD_DIM = 16, 4, 256
HD2 = HEAD_DIM // 2  # 128
ROPE_BASE = 10000.0
P = 128
QB = 512  # qs block width (moving free dim)
NSB = S // QB  # 4 s-blocks
NHT = HID // P  # 16 hidden chunks
NKS = S // P  # 16 key tiles
HPC = N_HEADS // 4  # 4 q heads per core
FQ = HPC * HEAD_DIM  # 1024 q features per core
NFQ = FQ // P  # 8 qT partition tiles
SCALE = 1.0 / math.sqrt(HEAD_DIM)

F32 = mybir.dt.float32
BF16 = mybir.dt.bfloat16
F8 = mybir.dt.float8e4
NP_BF16 = ml_dtypes.bfloat16
NP_F8 = ml_dtypes.float8_e4m3
F8MAX = 240.0
DR = mybir.MatmulPerfMode.DoubleRow
ACT_COPY = mybir.ActivationFunctionType.Copy

# power-of-2 quantization scales
S_HS = 32.0
S_W = 1024.0
S_QK = 16.0  # rope tables pre-scaled by S_QK / (S_HS * S_W) = 2^-11
S_V = 32.0
S_O = 32.0
EXP_SCALE = SCALE / (S_QK * S_QK)  # 2^-12
V_SCALE = S_V / (S_HS * S_W)  # 2^-10
OUT_SCALE = 1.0 / (S_O * S_W)  # 2^-15
MASK_HUGE = -1.0e15


def _build(mask_mode: str):
    """mask_mode: 'causal' | 'none' | 'full'. Returns compiled Bacc."""
    nc = bacc.Bacc("TRN2", target_bir_lowering=False, debug=False, num_devices=8)

    hsT_hi = nc.dram_tensor("hsT_hi", [HID, S], F8, kind="ExternalInput").ap()
    hsT_lo = nc.dram_tensor("hsT_lo", [HID, S], F8, kind="ExternalInput").ap()
    wq_hi = nc.dram_tensor("wq_hi", [HID, FQ], F8, kind="ExternalInput").ap()
    wq_lo = nc.dram_tensor("wq_lo", [HID, FQ], F8, kind="ExternalInput").ap()
    wk_hi = nc.dram_tensor("wk_hi", [HID, HEAD_DIM], F8, kind="ExternalInput").ap()
    wk_lo = nc.dram_tensor("wk_lo", [HID, HEAD_DIM], F8, kind="ExternalInput").ap()
    wv_hi = nc.dram_tensor("wv_hi", [HID, HEAD_DIM], F8, kind="ExternalInput").ap()
    wv_lo = nc.dram_tensor("wv_lo", [HID, HEAD_DIM], F8, kind="ExternalInput").ap()
    wo_hi = nc.dram_tensor("wo_hi", [FQ, HID], F8, kind="ExternalInput").ap()
    wo_lo = nc.dram_tensor("wo_lo", [FQ, HID], F8, kind="ExternalInput").ap()
    cosT = nc.dram_tensor("cosT", [HD2, S], BF16, kind="ExternalInput").ap()
    sinT = nc.dram_tensor("sinT", [HD2, S], BF16, kind="ExternalInput").ap()
    if mask_mode == "causal":
        tri01 = nc.dram_tensor("tri01", [P, P], BF16, kind="ExternalInput").ap()
    elif mask_mode == "full":
        ident = nc.dram_tensor("ident", [P, P], BF16, kind="ExternalInput").ap()
        maskT = nc.dram_tensor("maskT", [S, S], BF16, kind="ExternalInput").ap()
    out = nc.dram_tensor("out", [S, HID], BF16, kind="ExternalOutput").ap()

    with tile.TileContext(nc) as tc:
        with (
            tc.tile_pool(name="resid", bufs=1) as resid,
            tc.tile_pool(name="hst", bufs=1) as hst_pool,
            tc.tile_pool(name="oT", bufs=2) as oT_pool,
            tc.tile_pool(name="probs", bufs=6) as probs_pool,
            tc.tile_pool(name="tmp", bufs=1) as tmp_pool,
            tc.tile_pool(name="rb", bufs=1) as rb_pool,
            tc.tile_pool(name="mchunk", bufs=4) as mchunk_pool,
            tc.tile_pool(name="outsb", bufs=2) as outsb_pool,
            tc.tile_pool(name="ot", bufs=2) as ot_pool,
            tc.tile_pool(name="mm_ps", bufs=3, space="PSUM") as mm_ps,
            tc.tile_pool(name="sc_ps", bufs=3, space="PSUM") as sc_ps,
            tc.tile_pool(name="o_ps", bufs=2, space="PSUM") as o_ps,
            tc.tile_pool(name="sacc", bufs=2) as sacc_pool,
        ):
            # ---- persistent tiles (hi/lo fp8 pairs; dim1 = k-subtile) ----
            qT_hi = [resid.tile([P, 2, S], F8, tag=f"qTh{i}", name=f"qTh{i}") for i in range(HPC)]
            qT_lo = [resid.tile([P, 2, S], F8, tag=f"qTl{i}", name=f"qTl{i}") for i in range(HPC)]
            kT_hi = resid.tile([P, 2, S], F8, tag="kTh", name="kTh")
            kT_lo = resid.tile([P, 2, S], F8, tag="kTl", name="kTl")
            vt = resid.tile([P, NKS, HEAD_DIM], BF16, tag="vt", name="vt")
            cos_t = resid.tile([HD2, S], BF16, tag="cos", name="cos_t")
            sin_t = resid.tile([HD2, S], BF16, tag="sin", name="sin_t")
            wkh = resid.tile([P, NHT, HEAD_DIM], F8, tag="wkh", name="wkh")
            wkl = resid.tile([P, NHT, HEAD_DIM], F8, tag="wkl", name="wkl")
            wvh = resid.tile([P, NHT, HEAD_DIM], F8, tag="wvh", name="wvh")
            wvl = resid.tile([P, NHT, HEAD_DIM], F8, tag="wvl", name="wvl")
            wqh = resid.tile([P, NHT, FQ], F8, tag="wqh", name="wqh")
            wql = resid.tile([P, NHT, FQ], F8, tag="wql", name="wql")
            if mask_mode == "full":
                id_t = resid.tile([P, P], BF16, tag="ident", name="id_t")
                nc.sync.dma_start(out=id_t, in_=ident)
            if mask_mode == "causal":
                tri_t = resid.tile([P, P], BF16, tag="tri01", name="tri_t")
                nc.sync.dma_start(out=tri_t, in_=tri01)
            wo_h_r = resid.tile([P, NFQ, S], F8, tag="wohr", name="wo_h_r")
            wo_l_r = resid.tile([P, NFQ, S], F8, tag="wolr", name="wo_l_r")

            def dr3(ps, a_hi, a_lo, b_hi, b_lo, start, stop):
                """ps += a.T @ b over one K=256 slab (2 k-subtiles) via three
                fp8 DoubleRow matmuls, dropping the lo*lo term."""
                nc.tensor.matmul(ps, lhsT=a_hi, rhs=b_hi, start=start, stop=False, perf_mode=DR)
                nc.tensor.matmul(ps, lhsT=a_hi, rhs=b_lo, start=False, stop=False, perf_mode=DR)
                nc.tensor.matmul(ps, lhsT=a_lo, rhs=b_hi, start=False, stop=stop, perf_mode=DR)

            def rope_split(ps0, ps1, hiT, loT, sb):
                """RoPE + hi/lo fp8 split.  ps0/ps1: PSUM [P,QB] d-halves at
                scale S_HS*S_W; cos/sin pre-scaled so outputs land at S_QK.
                ACT evacuates PSUM to bf16 (frees the PSUM slot fast) and
                writes the fp8 hi parts; DVE does the bf16 arithmetic (SBUF
                2-byte fast path) and the fp8 lo parts."""
                cs = cos_t[:, sb * QB : (sb + 1) * QB]
                sn = sin_t[:, sb * QB : (sb + 1) * QB]
                c0 = tmp_pool.tile([P, QB], BF16, tag="c0", name="c0")
                c1 = tmp_pool.tile([P, QB], BF16, tag="c1", name="c1")
                nc.scalar.activation(c0, ps0, ACT_COPY)
                nc.scalar.activation(c1, ps1, ACT_COPY)
                t0 = tmp_pool.tile([P, QB], BF16, tag="t0", name="t0")
                t1 = tmp_pool.tile([P, QB], BF16, tag="t1", name="t1")
                t2 = tmp_pool.tile([P, QB], BF16, tag="t2", name="t2")
                t3 = tmp_pool.tile([P, QB], BF16, tag="t3", name="t3")
                d0 = tmp_pool.tile([P, QB], BF16, tag="d0", name="d0")
                d1 = tmp_pool.tile([P, QB], BF16, tag="d1", name="d1")
                nc.vector.tensor_mul(t0, c0, cs)
                nc.vector.tensor_mul(t3, c0, sn)
                nc.vector.tensor_mul(t1, c1, sn)
                nc.vector.tensor_mul(t2, c1, cs)
                sl = slice(sb * QB, (sb + 1) * QB)
                nc.vector.tensor_sub(d0, t0, t1)
                nc.scalar.activation(hiT[:, 0, sl], d0, ACT_COPY)
                nc.vector.tensor_sub(loT[:, 0, sl], d0, hiT[:, 0, sl])
                nc.vector.tensor_add(d1, t2, t3)
                nc.scalar.activation(hiT[:, 1, sl], d1, ACT_COPY)
                nc.vector.tensor_sub(loT[:, 1, sl], d1, hiT[:, 1, sl])

            # ================= phase 1: projections + rope =================
            for sb in range(NSB):
                ssl = slice(sb * QB, (sb + 1) * QB)
                hs_hi = hst_pool.tile([P, NHT, QB], F8, tag="hsh", name="hs_hi")
                hs_lo = hst_pool.tile([P, NHT, QB], F8, tag="hsl", name="hs_lo")
                for hq in range(4):
                    hsl4 = slice(4 * hq, 4 * hq + 4)
                    if sb == 0:
                        nc.sync.dma_start(
                            out=wkh[:, hsl4, :],
                            in_=wk_hi.rearrange("(t p) f -> p t f", p=P)[:, hsl4, :],
                        )
                        nc.sync.dma_start(
                            out=wkl[:, hsl4, :],
                            in_=wk_lo.rearrange("(t p) f -> p t f", p=P)[:, hsl4, :],
                        )
                    nc.sync.dma_start(
                        out=hs_hi[:, hsl4, :],
                        in_=hsT_hi.rearrange("(t p) s -> p t s", p=P)[:, hsl4, ssl],
                    )
                    nc.sync.dma_start(
                        out=hs_lo[:, hsl4, :],
                        in_=hsT_lo.rearrange("(t p) s -> p t s", p=P)[:, hsl4, ssl],
                    )
                # rope tables arrive as per-sb slices, keeping the startup
                # DMA window down to what the first kT/v matmuls need
                nc.sync.dma_start(out=cos_t[:, ssl], in_=cosT[:, ssl])
                nc.sync.dma_start(out=sin_t[:, ssl], in_=sinT[:, ssl])
                if sb == 0:
                    for hq in range(4):
                        hsl4 = slice(4 * hq, 4 * hq + 4)
                        nc.sync.dma_start(
                            out=wvh[:, hsl4, :],
                            in_=wv_hi.rearrange("(t p) f -> p t f", p=P)[:, hsl4, :],
                        )
                        nc.sync.dma_start(
                            out=wvl[:, hsl4, :],
                            in_=wv_lo.rearrange("(t p) f -> p t f", p=P)[:, hsl4, :],
                        )
                    # deferred bulk load: behind the sb0 essentials
                    for hq in range(4):
                        hsl4 = slice(4 * hq, 4 * hq + 4)
                        nc.sync.dma_start(
                            out=wqh[:, hsl4, :],
                            in_=wq_hi.rearrange("(t p) f -> p t f", p=P)[:, hsl4, :],
                        )
                        nc.sync.dma_start(
                            out=wql[:, hsl4, :],
                            in_=wq_lo.rearrange("(t p) f -> p t f", p=P)[:, hsl4, :],
                        )
                    # o_proj weights are block-invariant: load once, resident
                    for hq in range(4):
                        hslw = slice(hq * QB, (hq + 1) * QB)
                        nc.sync.dma_start(
                            out=wo_h_r[:, :, hslw],
                            in_=wo_hi.rearrange("(t p) h -> p t h", p=P)[:, :, hslw],
                        )
                        nc.sync.dma_start(
                            out=wo_l_r[:, :, hslw],
                            in_=wo_lo.rearrange("(t p) h -> p t h", p=P)[:, :, hslw],
                        )
                # k^T (one kv head: 2 d-halves), with rope
                ps_k = []
                for fd in range(2):
                    ps = mm_ps.tile([P, QB], F32, tag="mm", name="ps_mm")
                    fsl = slice(fd * P, (fd + 1) * P)
                    for tp in range(NHT // 2):
                        t2s = slice(2 * tp, 2 * tp + 2)
                        dr3(
                            ps,
                            wkh[:, t2s, fsl], wkl[:, t2s, fsl],
                            hs_hi[:, t2s, :], hs_lo[:, t2s, :],
                            start=(tp == 0), stop=(tp == NHT // 2 - 1),
                        )
                    ps_k.append(ps)
                rope_split(ps_k[0], ps_k[1], kT_hi, kT_lo, sb)
                # v (natural layout [s, d]), rescaled S_HS*S_W -> S_V at evac
                for s_sub in range(4):
                    pssl = slice(s_sub * P, (s_sub + 1) * P)
                    ps = mm_ps.tile([P, QB], F32, tag="mm", name="ps_mm")
                    for tp in range(NHT // 2):
                        t2s = slice(2 * tp, 2 * tp + 2)
                        dr3(
                            ps[:, :HEAD_DIM],
                            hs_hi[:, t2s, pssl], hs_lo[:, t2s, pssl],
                            wvh[:, t2s, :], wvl[:, t2s, :],
                            start=(tp == 0), stop=(tp == NHT // 2 - 1),
                        )
                    nc.scalar.activation(
                        vt[:, sb * 4 + s_sub, :], ps[:, :HEAD_DIM], ACT_COPY,
                        scale=V_SCALE,
                    )
                # q^T (4 heads x 2 d-halves), with rope
                for h in range(HPC):
                    ps_q = []
                    for fd in range(2):
                        ft = 2 * h + fd
                        fsl = slice(ft * P, (ft + 1) * P)
                        ps = mm_ps.tile([P, QB], F32, tag="mm", name="ps_mm")
                        for tp in range(NHT // 2):
                            t2s = slice(2 * tp, 2 * tp + 2)
                            dr3(
                                ps,
                                wqh[:, t2s, fsl], wql[:, t2s, fsl],
                                hs_hi[:, t2s, :], hs_lo[:, t2s, :],
                                start=(tp == 0), stop=(tp == NHT // 2 - 1),
                            )
                        ps_q.append(ps)
                    rope_split(ps_q[0], ps_q[1], qT_hi[h], qT_lo[h], sb)

            # ============ phase 2+3: attention + o_proj, per qs-block ============
            # o_proj for block qb is emitted AFTER attention for block qb+1
            # (software pipelining): by then the oT normalize chain
            # (rowsum -> recip -> split) has long drained, so the PE never
            # stalls at the o_proj matmuls.
            def o_proj_block(qb, oTh, oTl):
                for hc in range(NSB):
                    hsl = slice(hc * QB, (hc + 1) * QB)
                    for s_sub in range(4):
                        pssl = slice(s_sub * P, (s_sub + 1) * P)
                        ps = mm_ps.tile([P, QB], F32, tag="mm", name="ps_mm")
                        for fp_ in range(NFQ // 2):
                            f2s = slice(2 * fp_, 2 * fp_ + 2)
                            dr3(
                                ps,
                                oTh[:, f2s, pssl], oTl[:, f2s, pssl],
                                wo_h_r[:, f2s, hsl], wo_l_r[:, f2s, hsl],
                                start=(fp_ == 0), stop=(fp_ == NFQ // 2 - 1),
                            )
                        ot = outsb_pool.tile([P, QB], BF16, tag="ots", name="ots")
                        nc.scalar.activation(ot, ps, ACT_COPY, scale=OUT_SCALE)
                        nc.sync.dma_start(
                            out=out[qb * QB + s_sub * P : qb * QB + (s_sub + 1) * P, hsl],
                            in_=ot,
                        )

            pending_oproj = None
            for qb in range(NSB):
                qsl = slice(qb * QB, (qb + 1) * QB)
                nks = 4 * qb + 4 if mask_mode == "causal" else NKS
                oTh = oT_pool.tile([P, NFQ, QB], F8, tag="oTh", name="oTh")
                oTl = oT_pool.tile([P, NFQ, QB], F8, tag="oTl", name="oTl")
                for h in range(HPC):
                    ps_o0 = o_ps.tile([P, QB], F32, tag="o", name="ps_o")
                    ps_o1 = o_ps.tile([P, QB], F32, tag="o", name="ps_o")
                    acc = sacc_pool.tile([P, QB], F32, tag="acc", name="acc")

                    def pv(ks, c0, probs):
                        """PV matmuls for key-tile ks (probs live width [c0, QB));
                        emitted one iteration late so the exp of tile ks hides
                        behind the scores matmuls of tile ks+1."""
                        w = QB - c0
                        nc.tensor.matmul(
                            ps_o0[:, c0:],
                            lhsT=vt[:, ks, :HD2],
                            rhs=probs[:, :w],
                            start=(ks == 0),
                            stop=(ks == nks - 1),
                        )
                        nc.tensor.matmul(
                            ps_o1[:, c0:],
                            lhsT=vt[:, ks, HD2:],
                            rhs=probs[:, :w],
                            start=(ks == 0),
                            stop=(ks == nks - 1),
                        )

                    pv_q = []  # PV two iterations late: exp+sem latency stays hidden
                    for ks in range(nks):
                        ksl = slice(ks * P, (ks + 1) * P)
                        m = ks - 4 * qb if mask_mode == "causal" else -1
                        # columns [0, 128m) of a diagonal tile are fully
                        # masked -> compute only the live range [c0, QB)
                        c0 = 128 * m if m > 0 else 0
                        w = QB - c0
                        qslw = slice(qb * QB + c0, (qb + 1) * QB)
                        ps_s = sc_ps.tile([P, QB], F32, tag="sc", name="ps_s")
                        dr3(
                            ps_s[:, c0:],
                            kT_hi[:, :, ksl], kT_lo[:, :, ksl],
                            qT_hi[h][:, :, qslw], qT_lo[h][:, :, qslw],
                            start=True, stop=(mask_mode != "full"),
                        )
                        if mask_mode == "full":
                            mc = mchunk_pool.tile([P, QB], BF16, tag="mc", name="mc")
                            nc.sync.dma_start(out=mc, in_=maskT[ksl, qsl])
                            nc.tensor.matmul(
                                ps_s, lhsT=id_t, rhs=mc, start=False, stop=True
                            )
                        probs = probs_pool.tile([P, QB], BF16, tag="pr", name="probs")
                        nc.scalar.activation(
                            probs[:, :w],
                            ps_s[:, c0:],
                            mybir.ActivationFunctionType.Exp,
                            scale=EXP_SCALE,
                        )
                        if m >= 0:
                            # probs is stored offset (tile col 0 = abs col c0),
                            # so the diagonal 128-strip sits at tile cols [0,P):
                            # zero keys r > strip-col cc via the 0/1 triangle
                            nc.vector.tensor_mul(
                                probs[:, :P], probs[:, :P], tri_t
                            )
                        if len(pv_q) == 2:
                            pv(*pv_q.pop(0))
                        pv_q.append((ks, c0, probs))
                        if ks == 0:
                            nc.vector.tensor_copy(out=acc, in_=probs)
                        else:
                            nc.vector.tensor_add(
                                acc[:, c0:], acc[:, c0:], probs[:, :w]
                            )
                    for args in pv_q:
                        pv(*args)
                    # 1/colsum: partition all-reduce (broadcasts too), recip
                    zb = rb_pool.tile([P, QB], F32, tag="zb", name="zb")
                    nc.gpsimd.partition_all_reduce(
                        zb, acc, channels=P, reduce_op=bass_isa.ReduceOp.add
                    )
                    rb = rb_pool.tile([P, QB], F32, tag="rb", name="rb")
                    nc.vector.reciprocal_approx_fast(rb, zb)
                    # evacuate o^T (ACT frees PSUM), normalize, fp8 hi/lo split
                    for dh, ps_o in ((0, ps_o0), (1, ps_o1)):
                        f = 2 * h + dh
                        t = ot_pool.tile([P, QB], BF16, tag="t", name="t")
                        nc.scalar.activation(t, ps_o, ACT_COPY)
                        mm = ot_pool.tile([P, QB], BF16, tag="m", name="m")
                        nc.vector.tensor_mul(mm, t, rb)
                        nc.vector.tensor_copy(out=oTh[:, f, :], in_=mm)
                        nc.vector.tensor_sub(oTl[:, f, :], mm, oTh[:, f, :])
                if pending_oproj is not None:
                    o_proj_block(*pending_oproj)
                pending_oproj = (qb, oTh, oTl)
            o_proj_block(*pending_oproj)
    nc.compile()
    return nc


_BUILD_CACHE: dict = {}


def _get_kernel(mask_mode: str):
    if mask_mode not in _BUILD_CACHE:
        _BUILD_CACHE[mask_mode] = _build(mask_mode)
    return _BUILD_CACHE[mask_mode]


def _rope_tables(position_ids_b: np.ndarray):
    """cos/sin half-tables, transposed [HD2, S] bf16, pre-scaled by
    S_QK/(S_HS*S_W) so rope output lands at scale S_QK."""
    inv_freq = (
        1.0 / (ROPE_BASE ** (np.arange(0, HEAD_DIM, 2, dtype=np.float32) / HEAD_DIM))
    ).astype(np.float32)
    freqs = position_ids_b.astype(np.float32)[:, None] * inv_freq[None, :]  # [S, HD2]
    rs = np.float32(S_QK / (S_HS * S_W))
    return (
        np.ascontiguousarray((np.cos(freqs) * rs).T.astype(NP_BF16)),
        np.ascontiguousarray((np.sin(freqs) * rs).T.astype(NP_BF16)),
    )


def _split8(x: np.ndarray, scale: float):
    """x -> (hi, lo) e4m3 pair with x ~= (hi+lo)/scale."""
    xs = np.clip(x.astype(np.float32) * np.float32(scale), -F8MAX, F8MAX)
    hi = xs.astype(NP_F8)
    lo = np.clip(xs - hi.astype(np.float32), -F8MAX, F8MAX).astype(NP_F8)
    return np.ascontiguousarray(hi), np.ascontiguousarray(lo)


def kernel(hidden_states, attention_mask, position_ids, Wq, Wk, Wv, Wo):
    hidden_states = np.asarray(hidden_states, dtype=np.float32)
    attention_mask = np.asarray(attention_mask, dtype=np.float32)
    position_ids = np.asarray(position_ids)
    Wq = np.asarray(Wq, dtype=np.float32)
    Wk = np.asarray(Wk, dtype=np.float32)
    Wv = np.asarray(Wv, dtype=np.float32)
    Wo = np.asarray(Wo, dtype=np.float32)

    # mask classification
    tri = np.tril(np.ones((S, S), dtype=bool))
    canonical = np.where(tri, np.float32(0.0), np.float32(-1e9))
    is_causal = all(
        np.array_equal(attention_mask[b, 0], canonical) for b in range(B)
    )
    if is_causal:
        mask_mode = "causal"
    elif not attention_mask.any():
        mask_mode = "none"
    else:
        mask_mode = "full"

    nc = _get_kernel(mask_mode)

    if mask_mode == "causal":
        ii = np.arange(P)[:, None]
        jj = np.arange(P)[None, :]
        tri01 = np.where(jj >= ii, np.float32(1.0), np.float32(0.0)).astype(NP_BF16)

    in_maps = []
    for core in range(8):
        b, g = core // 4, core % 4
        hsT_hi, hsT_lo = _split8(hidden_states[b].T, S_HS)
        wq_hi, wq_lo = _split8(Wq[:, g * FQ : (g + 1) * FQ], S_W)
        wk_hi, wk_lo = _split8(Wk[:, g * HEAD_DIM : (g + 1) * HEAD_DIM], S_W)
        wv_hi, wv_lo = _split8(Wv[:, g * HEAD_DIM : (g + 1) * HEAD_DIM], S_W)
        wo_hi, wo_lo = _split8(Wo[g * FQ : (g + 1) * FQ, :], S_W)
        m = {
            "hsT_hi": hsT_hi, "hsT_lo": hsT_lo,
            "wq_hi": wq_hi, "wq_lo": wq_lo,
            "wk_hi": wk_hi, "wk_lo": wk_lo,
            "wv_hi": wv_hi, "wv_lo": wv_lo,
            "wo_hi": wo_hi, "wo_lo": wo_lo,
        }
        cosT, sinT = _rope_tables(position_ids[b])
        m["cosT"], m["sinT"] = cosT, sinT
        if mask_mode == "causal":
            m["tri01"] = tri01
        elif mask_mode == "full":
            m["ident"] = np.eye(P, dtype=np.float32).astype(NP_BF16)
            m["maskT"] = np.ascontiguousarray(
                (attention_mask[b, 0].T * np.float32(S_QK * S_QK))
            ).astype(NP_BF16)
        in_maps.append(m)

    global _LAST_IN_MAPS
    _LAST_IN_MAPS = in_maps
    res = run_bass_kernel_spmd(nc, in_maps, list(range(8)))
    outs = [res.results[c]["out"].astype(np.float32) for c in range(8)]
    full = np.empty((B, S, HID), dtype=np.float32)
    for b in range(B):
        full[b] = outs[4 * b] + outs[4 * b + 1] + outs[4 * b + 2] + outs[4 * b + 3]
    return full
